# revision 10
# baseline (speedup 1.0000x reference)
"""Trainium2 Bass kernel for nn_DenseGraphWaveletLayer (v6).

out[:, l, :] = phi_l @ diag(theta) @ phi_inv_l @ (features[:, l, :] @ W)

v6 strategy (8 cores SPMD, one program, per-core data):
  - Both spmms are HBM dma_gather (transpose=False, 256B rows) + one-hot
    sel matmuls. spmm1 gathers bf16 feature rows; its edge values carry
    diag(theta) folded on host. spmm2 gathers bf16 z rows from the
    AllGather'd shared-DRAM z table.
  - sel is built per 64-row WINDOW (each 128-row block = 2 windows, with
    slots row-sorted and window-partitioned on host), halving the DVE
    one-hot build: sel = (rl' == iota64) * val over [slot, 64] only.
    Matmuls write psU[:, w*64:(w+1)*64] (spmm1) / psO[w*64:.., :] (spmm2)
    as two independent accumulation chains per block.
  - Slot order per (l, mat): half-major (lo run, hi run); gathers span
    blocks in chunks of up to DGW_CAP idxs on 4 round-robin SWDGE queues,
    emitted balanced by per-half block coverage.
  - aux (rl'/val) loaded once per (l, mat); idx loaded per chunk; pads use
    rl'=200/val=0 over a valid dup token so everything stays finite.
"""

import os
import sys
import types

import numpy as np
import ml_dtypes

BF16 = ml_dtypes.bfloat16

N = 50000
L = 4
C = 128
NCORES = 8
BLK = 128
NB_TOT = (N + BLK - 1) // BLK            # 391
NBPC = (NB_TOT + NCORES - 1) // NCORES   # 49
SHARD = NBPC * BLK                       # 6272
TBL = NCORES * SHARD                     # 50176
HALF = 32768
W = 64                                   # sel row-window width
CAP = int(os.environ.get("DGW_CAP", 4096))
KSEL = int(os.environ.get("DGW_KSEL", 16))
NQUEUES = 4
MINCNT = 16


def _install_hook_stub():
    try:
        import antenv
    except ImportError:
        return
    try:
        from antenv import axon_hooks  # noqa: F401
        return
    except ImportError:
        pass
    mod = types.ModuleType("antenv.axon_hooks")
    mod._hook = None
    mod.set_axon_ntff_profile_hook = lambda h: setattr(mod, "_hook", h)
    mod.get_axon_ntff_profile_hook = lambda: mod._hook
    sys.modules["antenv.axon_hooks"] = mod
    antenv.axon_hooks = mod


def _tok2(cols):
    """z-table token id for spmm2: row (core*128 + rl) * NBPC + k."""
    cblk = cols >> 7
    rl = cols & 127
    core = cblk % NCORES
    k = cblk // NCORES
    return (core * 128 + rl) * NBPC + k


class MatPlan:
    """Per-(l, half, block, win) slot layout for one sparse matrix."""

    def __init__(self):
        self.slots = None    # [L, 2, NBPC, 2] padded counts (mult of 128)
        self.segoff = None   # [L, 2, NBPC, 2] slot offset of sub-segment
        self.auxoff = None   # [L, 2, NBPC, 2] aux col offset
        self.nt = None       # [L, 2, NBPC, 2] tiles per sub-segment
        self.tot_slots = 0
        self.tot_aux = 0


def _preprocess_mat(rows_l, ckey_l, vals_l):
    """rows/ckey/vals: lists of L arrays (full edge sets).

    Returns (plan, idx [8,128,S/16], aux [8,128,A])."""
    plan = MatPlan()
    NW = 2
    cnt = np.zeros((L, NCORES, 2, NBPC, NW), np.int64)
    per_l = []
    for l in range(L):
        rows, ckey, vals = rows_l[l], ckey_l[l], vals_l[l]
        core = (rows >> 7) % NCORES
        k = rows >> 10
        rl = (rows & 127).astype(np.int64)
        w = rl >> 6
        hi = (ckey >= HALF).astype(np.int64)
        idxv = (ckey - HALF * hi).astype(np.int16)
        g = (((core * 2 + hi) * NBPC + k) * NW + w).astype(np.int64)
        cnt[l] = np.bincount(g, minlength=NCORES * 2 * NBPC * NW).reshape(
            NCORES, 2, NBPC, NW)
        per_l.append((g, idxv, (rl & 63).astype(np.int16),
                      vals.astype(np.float32)))

    maxc = np.maximum(cnt.max(axis=1), MINCNT)          # [L, 2, NBPC, NW]
    plan.slots = ((maxc + 127) // 128) * 128
    plan.nt = plan.slots // 128
    flat = plan.slots.reshape(-1)
    off = np.concatenate(([0], np.cumsum(flat)[:-1]))
    plan.segoff = off.reshape(L, 2, NBPC, NW)
    aux_b = 2 * plan.nt
    aoff = np.concatenate(([0], np.cumsum(aux_b.reshape(-1))[:-1]))
    plan.auxoff = aoff.reshape(L, 2, NBPC, NW)
    plan.tot_slots = int(flat.sum())
    plan.tot_aux = int(aux_b.sum())

    S = plan.tot_slots
    # pad slots gather a valid token (idx 0) killed by rl'=200 -> sel=0.
    idx_flat = np.zeros((NCORES, S), np.int16)
    rl_flat = np.full((NCORES, S), 200, np.int16)
    val_flat = np.zeros((NCORES, S), np.float32)

    for l in range(L):
        g, idxv, rlw, vals = per_l[l]
        order = np.argsort(g, kind="stable")
        g_s = g[order]
        grp_cnt = cnt[l].reshape(-1)
        starts = np.concatenate(([0], np.cumsum(grp_cnt)[:-1]))
        rank = np.arange(len(order)) - starts[g_s]
        c_s = g_s // (2 * NBPC * NW)
        hi_s = (g_s // (NBPC * NW)) % 2
        k_s = (g_s // NW) % NBPC
        w_s = g_s % NW
        slot = plan.segoff[l, hi_s, k_s, w_s] + rank
        idx_flat[c_s, slot] = idxv[order]
        rl_flat[c_s, slot] = rlw[order]
        val_flat[c_s, slot] = vals[order]

    idx_w = np.ascontiguousarray(np.tile(
        idx_flat.reshape(NCORES, S // 16, 16).transpose(0, 2, 1), (1, 8, 1)))

    A = plan.tot_aux
    aux = np.zeros((NCORES, 128, A), np.float32)
    for l in range(L):
        for h in range(2):
            for k in range(NBPC):
                for w in range(NW):
                    o = plan.segoff[l, h, k, w]
                    ao = plan.auxoff[l, h, k, w]
                    nt = plan.nt[l, h, k, w]
                    s = slice(o, o + nt * 128)
                    aux[:, :, ao:ao + nt] = rl_flat[:, s].reshape(
                        NCORES, nt, 128).transpose(0, 2, 1)
                    aux[:, :, ao + nt:ao + 2 * nt] = val_flat[:, s].reshape(
                        NCORES, nt, 128).transpose(0, 2, 1)
    return plan, idx_w, np.ascontiguousarray(aux.astype(BF16))


def _preprocess(phi_indices, phi_values, phi_inverse_indices,
                phi_inverse_values, diagonal_weight_filter):
    diag = np.asarray(diagonal_weight_filter, np.float64)
    rows1, ck1, v1 = [], [], []
    rows2, ck2, v2 = [], [], []
    for l in range(L):
        r1 = phi_inverse_indices[l, 0].astype(np.int64)
        c1 = phi_inverse_indices[l, 1].astype(np.int64)
        rows1.append(r1)
        ck1.append(c1)
        v1.append((np.asarray(phi_inverse_values[l], np.float64) * diag[r1]
                   ).astype(np.float32))
        r2 = phi_indices[l, 0].astype(np.int64)
        c2 = phi_indices[l, 1].astype(np.int64)
        rows2.append(r2)
        ck2.append(_tok2(c2))
        v2.append(np.asarray(phi_values[l], np.float32))
    plan1, idx1, aux1 = _preprocess_mat(rows1, ck1, v1)
    plan2, idx2, aux2 = _preprocess_mat(rows2, ck2, v2)
    return plan1, idx1, aux1, plan2, idx2, aux2


def _build(plan1, plan2, scales):
    import concourse.mybir as mybir
    import concourse.tile as tile
    from concourse import bacc

    f32 = mybir.dt.float32
    bf16 = mybir.dt.bfloat16
    i16 = mybir.dt.int16
    eq = mybir.AluOpType.is_equal
    mult = mybir.AluOpType.mult
    AF = mybir.ActivationFunctionType

    nc = bacc.Bacc("TRN2", target_bir_lowering=False, debug=False,
                   num_devices=NCORES, num_swdge_queues=NQUEUES)
    featsB = nc.dram_tensor("featsB", [L * N, C], bf16, kind="ExternalInput")
    wmat = nc.dram_tensor("wmat", [C, C], bf16, kind="ExternalInput")
    iotaT = nc.dram_tensor("iotaT", [128, KSEL * W], bf16,
                           kind="ExternalInput")
    idx1 = nc.dram_tensor("idx1", [128, plan1.tot_slots // 16], i16,
                          kind="ExternalInput")
    idx2 = nc.dram_tensor("idx2", [128, plan2.tot_slots // 16], i16,
                          kind="ExternalInput")
    aux1 = nc.dram_tensor("aux1", [128, plan1.tot_aux], bf16,
                          kind="ExternalInput")
    aux2 = nc.dram_tensor("aux2", [128, plan2.tot_aux], bf16,
                          kind="ExternalInput")
    outp = nc.dram_tensor("outp", [L, SHARD, C], f32, kind="ExternalOutput")

    qn = [0]

    def q():
        qn[0] += 1
        return qn[0] % NQUEUES

    with tile.TileContext(nc) as tc:
        with (
            tc.tile_pool(name="const", bufs=1) as constp,
            tc.tile_pool(name="aux", bufs=3) as auxp,
            tc.tile_pool(name="idx", bufs=8) as idxp,
            tc.tile_pool(name="dst1", bufs=6) as dst1p,
            tc.tile_pool(name="dst2", bufs=6) as dst2p,
            tc.tile_pool(name="sel", bufs=6) as selp,
            tc.tile_pool(name="stg", bufs=4) as stgp,
            tc.tile_pool(name="zsb", bufs=2) as zsbp,
            tc.tile_pool(name="psU", bufs=2, space="PSUM") as psUp,
            tc.tile_pool(name="psZ", bufs=1, space="PSUM") as psZp,
            tc.tile_pool(name="psO", bufs=2, space="PSUM") as psOp,
            tc.tile_pool(name="dram", bufs=4, space="DRAM") as dramp,
        ):
            w_t = constp.tile([C, C], bf16, name="w_t")
            nc.sync.dma_start(w_t[:], wmat[:])
            io_t = constp.tile([128, KSEL * W], bf16, name="io_t")
            nc.sync.dma_start(io_t[:], iotaT[:])

            ztbs = []

            def emit_mat(plan, idxT, auxT, dstp, l, srcs, consume_block):
                """Gathers (chunk-interleaved lo/hi, balanced by block
                coverage) + per-block matmul consumption for one (l, mat)."""
                ao0 = int(plan.auxoff[l, 0, 0, 0])
                a_end = (int(plan.auxoff[l, 1, NBPC - 1, 1])
                         + 2 * int(plan.nt[l, 1, NBPC - 1, 1]))
                vt = auxp.tile([128, a_end - ao0], bf16, tag="aux")
                nc.sync.dma_start(vt[:], auxT[:, ao0:a_end])

                QCAP = 4 * CAP
                runs = []
                for h in range(2):
                    s0 = int(plan.segoff[l, h, 0, 0])
                    ln = (int(plan.segoff[l, h, NBPC - 1, 1])
                          + int(plan.slots[l, h, NBPC - 1, 1]) - s0)
                    chunks = []
                    o = 0
                    while o < ln:
                        n = min(CAP, ln - o)
                        chunks.append((s0 + o, n))
                        o += n
                    runs.append(chunks)
                itiles = {}         # (h, qi) -> idx tile

                def idx_tile(h, i, s0, n):
                    # one idx tile covers 4 consecutive chunks (QCAP slots)
                    qi = i // 4
                    if (h, qi) not in itiles:
                        run0 = int(plan.segoff[l, h, 0, 0])
                        qs0 = run0 + qi * QCAP
                        ln = (int(plan.segoff[l, h, NBPC - 1, 1])
                              + int(plan.slots[l, h, NBPC - 1, 1]) - run0)
                        qn = min(QCAP, run0 + ln - qs0)
                        it = idxp.tile([128, QCAP // 16], i16, tag="idx")
                        nc.sync.dma_start(
                            it[:, :qn // 16],
                            idxT[:, qs0 // 16:(qs0 + qn) // 16])
                        itiles[(h, qi)] = (it, qs0)
                    it, qs0 = itiles[(h, qi)]
                    return it[:, (s0 - qs0) // 16:(s0 - qs0 + n) // 16]

                tiles = {}          # (h, i) -> dst tile
                covered = [0, 0]    # absolute slot end gathered per half
                nextb = [0]

                def emit_ready():
                    while nextb[0] < NBPC:
                        b = nextb[0]
                        ends = [int(plan.segoff[l, h, b, 1])
                                + int(plan.slots[l, h, b, 1])
                                for h in range(2)]
                        if covered[0] < ends[0] or covered[1] < ends[1]:
                            return
                        # tiles of this block in (w, h, t) order so each
                        # window forms one contiguous accumulation chain
                        tl = []
                        for w in range(2):
                            for h in range(2):
                                seg0 = int(plan.segoff[l, h, b, w])
                                nt = int(plan.nt[l, h, b, w])
                                run0 = int(plan.segoff[l, h, 0, 0])
                                for t in range(nt):
                                    s = seg0 + t * 128 - run0
                                    ti, loc = s // CAP, (s % CAP) // 128
                                    tl.append((tiles[(h, ti)], loc,
                                               (h, b, w, t)))
                        consume_block(b, tl, vt, ao0)
                        nextb[0] += 1

                def blocks_done(h):
                    bc = 0
                    while bc < NBPC and (int(plan.segoff[l, h, bc, 1])
                                         + int(plan.slots[l, h, bc, 1])
                                         <= covered[h]):
                        bc += 1
                    return bc

                ci = [0, 0]
                while ci[0] < len(runs[0]) or ci[1] < len(runs[1]):
                    if ci[0] >= len(runs[0]):
                        h = 1
                    elif ci[1] >= len(runs[1]):
                        h = 0
                    else:
                        h = 0 if blocks_done(0) <= blocks_done(1) else 1
                    s0, n = runs[h][ci[h]]
                    dst = dstp.tile([128, CAP // 128, C], bf16, tag="dst")
                    iv = idx_tile(h, ci[h], s0, n)
                    if ci[h] % 4 == 2 and ci[h] + 2 < len(runs[h]):
                        ps0, pn = runs[h][ci[h] + 2]
                        idx_tile(h, ci[h] + 2, ps0, pn)
                    nc.gpsimd.dma_gather(
                        dst[:, :n // 128, :], srcs[h],
                        iv, n, n, C,
                        single_packet=False, queue_num=q())
                    tiles[(h, ci[h])] = dst
                    covered[h] = s0 + n
                    ci[h] += 1
                    emit_ready()
                emit_ready()
                assert nextb[0] == NBPC

            def sel_groups(tl, vt, ao0, plan, l):
                """Yield (sel, k, tile, loc, w, first, last): sel columns
                [k*W:(k+1)*W] for consecutive tiles of one sub-segment."""
                nw = [0, 0]
                for _, _, (h, b, w, t) in tl:
                    nw[w] += 1
                done = [0, 0]
                i = 0
                while i < len(tl):
                    h, b, w, t0 = tl[i][2]
                    nt = int(plan.nt[l, h, b, w])
                    K = min(KSEL, nt - t0)
                    ao = int(plan.auxoff[l, h, b, w]) - ao0
                    sel = selp.tile([128, KSEL * W], bf16, tag="sel")
                    s3 = sel[:, :K * W].rearrange("p (k r) -> p k r", k=K)
                    rl_b = vt[:, ao + t0:ao + t0 + K].to_broadcast(
                        [128, K, W])
                    io_v = io_t[:, :K * W].rearrange(
                        "p (k r) -> p k r", k=K)
                    nc.vector.tensor_tensor(out=s3, in0=rl_b, in1=io_v,
                                            op=eq)
                    v_b = vt[:, ao + nt + t0:ao + nt + t0 + K
                             ].to_broadcast([128, K, W])
                    nc.vector.tensor_tensor(out=s3, in0=s3, in1=v_b,
                                            op=mult)
                    for k in range(K):
                        yield (sel, k, tl[i + k][0], tl[i + k][1], w,
                               done[w] == 0, done[w] == nw[w] - 1)
                        done[w] += 1
                    i += K

            def spmm1_scale(l):
                zshw = dramp.tile([128, NBPC * C], bf16, tag="zshw")
                ztbw = dramp.tile([TBL, C], bf16, tag="ztbw",
                                  addr_space="Shared")
                ztbs.append(ztbw)
                zsb = zsbp.tile([128, NBPC * C], bf16, tag="zsb")
                src_lo = featsB[l * N:l * N + HALF, :]
                src_hi = featsB[l * N + HALF:(l + 1) * N, :]

                def consume(b, tl, vt, ao0):
                    psU = psUp.tile([128, 128], f32)
                    for sel, k, dt, loc, w, first, last in sel_groups(
                            tl, vt, ao0, plan1, l):
                        nc.tensor.matmul(
                            psU[:, w * W:(w + 1) * W], lhsT=dt[:, loc, :],
                            rhs=sel[:, k * W:(k + 1) * W],
                            start=first, stop=last)
                    ut = stgp.tile([128, 128], bf16, tag="ut")
                    nc.scalar.activation(ut[:], psU[:], AF.Copy)
                    psZ = psZp.tile([128, 128], f32)
                    nc.tensor.matmul(psZ[:], lhsT=ut[:], rhs=w_t[:],
                                     start=True, stop=True)
                    nc.scalar.activation(zsb[:, b * C:(b + 1) * C], psZ[:],
                                         AF.Copy)

                emit_mat(plan1, idx1, aux1, dst1p, l, (src_lo, src_hi),
                         consume)
                nc.scalar.dma_start(zshw[:], zsb[:])
                nc.gpsimd.collective_compute(
                    "AllGather", mybir.AluOpType.bypass,
                    replica_groups=[list(range(NCORES))],
                    ins=[zshw.opt()], outs=[ztbw.opt()])

            def spmm2_scale(l):
                ztbw = ztbs[l]
                src_lo = ztbw[:HALF, :]
                src_hi = ztbw[HALF:, :]

                def consume(b, tl, vt, ao0):
                    psO = psOp.tile([128, 128], f32)
                    for sel, k, dt, loc, w, first, last in sel_groups(
                            tl, vt, ao0, plan2, l):
                        nc.tensor.matmul(
                            psO[w * W:(w + 1) * W, :],
                            lhsT=sel[:, k * W:(k + 1) * W],
                            rhs=dt[:, loc, :],
                            start=first, stop=last)
                    ot = stgp.tile([128, 128], f32, tag="ot")
                    nc.vector.tensor_copy(ot[:], psO[:])
                    nc.scalar.dma_start(outp[l, b * BLK:(b + 1) * BLK, :],
                                        ot[:])

                emit_mat(plan2, idx2, aux2, dst2p, l, (src_lo, src_hi),
                         consume)

            done1 = done2 = 0
            while done2 < scales:
                if done1 < scales:
                    spmm1_scale(done1)
                    done1 += 1
                if done1 >= min(2, scales) and done2 < done1:
                    spmm2_scale(done2)
                    done2 += 1
    nc.compile()
    return nc


def kernel(**inputs):
    _install_hook_stub()
    from concourse.bass_utils import run_bass_kernel_spmd

    feats = np.asarray(inputs["features"], np.float32)        # [N, L, C]
    featsB = np.ascontiguousarray(
        feats.transpose(1, 0, 2).reshape(L * N, C)).astype(BF16)
    wmat = np.asarray(inputs["weight_matrix"], np.float32).astype(BF16)

    plan1, idx1, aux1, plan2, idx2, aux2 = _preprocess(
        np.asarray(inputs["phi_indices"]), np.asarray(inputs["phi_values"]),
        np.asarray(inputs["phi_inverse_indices"]),
        np.asarray(inputs["phi_inverse_values"]),
        np.asarray(inputs["diagonal_weight_filter"]))

    scales = int(os.environ.get("DGW_SCALES", L))
    nc = _build(plan1, plan2, scales)

    iotaT = np.ascontiguousarray(np.tile(
        np.arange(W, dtype=np.float32)[None, :], (128, KSEL))).astype(BF16)

    in_maps = []
    for c in range(NCORES):
        in_maps.append(dict(
            featsB=featsB, wmat=wmat, iotaT=iotaT,
            idx1=np.ascontiguousarray(idx1[c]),
            idx2=np.ascontiguousarray(idx2[c]),
            aux1=np.ascontiguousarray(aux1[c]),
            aux2=np.ascontiguousarray(aux2[c])))
    res = run_bass_kernel_spmd(nc, in_maps, core_ids=list(range(NCORES)))
    kernel.last_results = res

    shards = np.stack([res.results[c]["outp"] for c in range(NCORES)])
    blocks = shards.reshape(NCORES, L, NBPC, BLK, C).transpose(2, 0, 3, 1, 4)
    out = blocks.reshape(NBPC * NCORES * BLK, L, C)[:N]
    return np.ascontiguousarray(out)


# revision 11
# speedup vs baseline: 1.0501x; 1.0501x over previous
"""Trainium2 Bass kernel for nn_DenseGraphWaveletLayer (v6).

out[:, l, :] = phi_l @ diag(theta) @ phi_inv_l @ (features[:, l, :] @ W)

v6 strategy (8 cores SPMD, one program, per-core data):
  - Both spmms are HBM dma_gather (transpose=False, 256B rows) + one-hot
    sel matmuls. spmm1 gathers bf16 feature rows; its edge values carry
    diag(theta) folded on host. spmm2 gathers bf16 z rows from the
    AllGather'd shared-DRAM z table.
  - sel is built per 64-row WINDOW (each 128-row block = 2 windows, with
    slots row-sorted and window-partitioned on host), halving the DVE
    one-hot build: sel = (rl' == iota64) * val over [slot, 64] only.
    Matmuls write psU[:, w*64:(w+1)*64] (spmm1) / psO[w*64:.., :] (spmm2)
    as two independent accumulation chains per block.
  - Slot order per (l, mat): half-major (lo run, hi run); gathers span
    blocks in chunks of up to DGW_CAP idxs on 4 round-robin SWDGE queues,
    emitted balanced by per-half block coverage.
  - aux (rl'/val) loaded once per (l, mat); idx loaded per chunk; pads use
    rl'=200/val=0 over a valid dup token so everything stays finite.
"""

import os
import sys
import types

import numpy as np
import ml_dtypes

BF16 = ml_dtypes.bfloat16

N = 50000
L = 4
C = 128
NCORES = 8
BLK = 128
NB_TOT = (N + BLK - 1) // BLK            # 391
NBPC = (NB_TOT + NCORES - 1) // NCORES   # 49
SHARD = NBPC * BLK                       # 6272
TBL = NCORES * SHARD                     # 50176
HALF = 32768
W = 64                                   # sel row-window width
CAP = int(os.environ.get("DGW_CAP", 4096))
KSEL = int(os.environ.get("DGW_KSEL", 16))
NQUEUES = 4
MINCNT = 16


def _install_hook_stub():
    try:
        import antenv
    except ImportError:
        return
    try:
        from antenv import axon_hooks  # noqa: F401
        return
    except ImportError:
        pass
    mod = types.ModuleType("antenv.axon_hooks")
    mod._hook = None
    mod.set_axon_ntff_profile_hook = lambda h: setattr(mod, "_hook", h)
    mod.get_axon_ntff_profile_hook = lambda: mod._hook
    sys.modules["antenv.axon_hooks"] = mod
    antenv.axon_hooks = mod


def _tok2(cols):
    """z-table token id for spmm2: row (core*128 + rl) * NBPC + k."""
    cblk = cols >> 7
    rl = cols & 127
    core = cblk % NCORES
    k = cblk // NCORES
    return (core * 128 + rl) * NBPC + k


class MatPlan:
    """Per-(l, half, block, win) slot layout for one sparse matrix."""

    def __init__(self):
        self.slots = None    # [L, 2, NBPC, 2] padded counts (mult of 128)
        self.segoff = None   # [L, 2, NBPC, 2] slot offset of sub-segment
        self.auxoff = None   # [L, 2, NBPC, 2] aux col offset
        self.nt = None       # [L, 2, NBPC, 2] tiles per sub-segment
        self.tot_slots = 0
        self.tot_aux = 0


def _preprocess_mat(rows_l, ckey_l, vals_l):
    """rows/ckey/vals: lists of L arrays (full edge sets).

    Returns (plan, idx [8,128,S/16], aux [8,128,A])."""
    plan = MatPlan()
    NW = 2
    cnt = np.zeros((L, NCORES, 2, NBPC, NW), np.int64)
    per_l = []
    for l in range(L):
        rows, ckey, vals = rows_l[l], ckey_l[l], vals_l[l]
        core = (rows >> 7) % NCORES
        k = rows >> 10
        rl = (rows & 127).astype(np.int64)
        w = rl >> 6
        hi = (ckey >= HALF).astype(np.int64)
        idxv = (ckey - HALF * hi).astype(np.int16)
        g = (((core * 2 + hi) * NBPC + k) * NW + w).astype(np.int64)
        cnt[l] = np.bincount(g, minlength=NCORES * 2 * NBPC * NW).reshape(
            NCORES, 2, NBPC, NW)
        per_l.append((g, idxv, (rl & 63).astype(np.int16),
                      vals.astype(np.float32)))

    maxc = np.maximum(cnt.max(axis=1), MINCNT)          # [L, 2, NBPC, NW]
    plan.slots = ((maxc + 127) // 128) * 128
    plan.nt = plan.slots // 128
    flat = plan.slots.reshape(-1)
    off = np.concatenate(([0], np.cumsum(flat)[:-1]))
    plan.segoff = off.reshape(L, 2, NBPC, NW)
    aux_b = 2 * plan.nt
    aoff = np.concatenate(([0], np.cumsum(aux_b.reshape(-1))[:-1]))
    plan.auxoff = aoff.reshape(L, 2, NBPC, NW)
    plan.tot_slots = int(flat.sum())
    plan.tot_aux = int(aux_b.sum())

    S = plan.tot_slots
    # pad slots gather a valid token (idx 0) killed by rl'=200 -> sel=0.
    idx_flat = np.zeros((NCORES, S), np.int16)
    rl_flat = np.full((NCORES, S), 200, np.int16)
    val_flat = np.zeros((NCORES, S), np.float32)

    for l in range(L):
        g, idxv, rlw, vals = per_l[l]
        order = np.argsort(g, kind="stable")
        g_s = g[order]
        grp_cnt = cnt[l].reshape(-1)
        starts = np.concatenate(([0], np.cumsum(grp_cnt)[:-1]))
        rank = np.arange(len(order)) - starts[g_s]
        c_s = g_s // (2 * NBPC * NW)
        hi_s = (g_s // (NBPC * NW)) % 2
        k_s = (g_s // NW) % NBPC
        w_s = g_s % NW
        slot = plan.segoff[l, hi_s, k_s, w_s] + rank
        idx_flat[c_s, slot] = idxv[order]
        rl_flat[c_s, slot] = rlw[order]
        val_flat[c_s, slot] = vals[order]

    idx_w = np.ascontiguousarray(np.tile(
        idx_flat.reshape(NCORES, S // 16, 16).transpose(0, 2, 1), (1, 8, 1)))

    A = plan.tot_aux
    aux = np.zeros((NCORES, 128, A), np.float32)
    for l in range(L):
        for h in range(2):
            for k in range(NBPC):
                for w in range(NW):
                    o = plan.segoff[l, h, k, w]
                    ao = plan.auxoff[l, h, k, w]
                    nt = plan.nt[l, h, k, w]
                    s = slice(o, o + nt * 128)
                    aux[:, :, ao:ao + nt] = rl_flat[:, s].reshape(
                        NCORES, nt, 128).transpose(0, 2, 1)
                    aux[:, :, ao + nt:ao + 2 * nt] = val_flat[:, s].reshape(
                        NCORES, nt, 128).transpose(0, 2, 1)
    return plan, idx_w, np.ascontiguousarray(aux.astype(BF16))


def _preprocess(phi_indices, phi_values, phi_inverse_indices,
                phi_inverse_values, diagonal_weight_filter):
    diag = np.asarray(diagonal_weight_filter, np.float64)
    rows1, ck1, v1 = [], [], []
    rows2, ck2, v2 = [], [], []
    for l in range(L):
        r1 = phi_inverse_indices[l, 0].astype(np.int64)
        c1 = phi_inverse_indices[l, 1].astype(np.int64)
        rows1.append(r1)
        ck1.append(c1)
        v1.append((np.asarray(phi_inverse_values[l], np.float64) * diag[r1]
                   ).astype(np.float32))
        r2 = phi_indices[l, 0].astype(np.int64)
        c2 = phi_indices[l, 1].astype(np.int64)
        rows2.append(r2)
        ck2.append(_tok2(c2))
        v2.append(np.asarray(phi_values[l], np.float32))
    plan1, idx1, aux1 = _preprocess_mat(rows1, ck1, v1)
    plan2, idx2, aux2 = _preprocess_mat(rows2, ck2, v2)
    return plan1, idx1, aux1, plan2, idx2, aux2


def _build(plan1, plan2, scales):
    import concourse.mybir as mybir
    import concourse.tile as tile
    from concourse import bacc

    f32 = mybir.dt.float32
    bf16 = mybir.dt.bfloat16
    i16 = mybir.dt.int16
    eq = mybir.AluOpType.is_equal
    mult = mybir.AluOpType.mult
    AF = mybir.ActivationFunctionType

    nc = bacc.Bacc("TRN2", target_bir_lowering=False, debug=False,
                   num_devices=NCORES, num_swdge_queues=NQUEUES,
                   dynamic_dma_scratch_size=int(
                       os.environ.get("DGW_RING", 49152)))
    featsB = nc.dram_tensor("featsB", [L * N, C], bf16, kind="ExternalInput")
    wmat = nc.dram_tensor("wmat", [C, C], bf16, kind="ExternalInput")
    iotaT = nc.dram_tensor("iotaT", [128, KSEL * W], bf16,
                           kind="ExternalInput")
    idx1 = nc.dram_tensor("idx1", [128, plan1.tot_slots // 16], i16,
                          kind="ExternalInput")
    idx2 = nc.dram_tensor("idx2", [128, plan2.tot_slots // 16], i16,
                          kind="ExternalInput")
    aux1 = nc.dram_tensor("aux1", [128, plan1.tot_aux], bf16,
                          kind="ExternalInput")
    aux2 = nc.dram_tensor("aux2", [128, plan2.tot_aux], bf16,
                          kind="ExternalInput")
    outp = nc.dram_tensor("outp", [L, SHARD, C], f32, kind="ExternalOutput")

    qn = [0]

    def q():
        qn[0] += 1
        return qn[0] % NQUEUES

    with tile.TileContext(nc) as tc:
        with (
            tc.tile_pool(name="const", bufs=1) as constp,
            tc.tile_pool(name="aux", bufs=3) as auxp,
            tc.tile_pool(name="idx", bufs=8) as idxp,
            tc.tile_pool(name="dst1", bufs=6) as dst1p,
            tc.tile_pool(name="dst2", bufs=6) as dst2p,
            tc.tile_pool(name="sel", bufs=6) as selp,
            tc.tile_pool(name="stg", bufs=4) as stgp,
            tc.tile_pool(name="zsb", bufs=2) as zsbp,
            tc.tile_pool(name="psU", bufs=2, space="PSUM") as psUp,
            tc.tile_pool(name="psZ", bufs=1, space="PSUM") as psZp,
            tc.tile_pool(name="psO", bufs=2, space="PSUM") as psOp,
            tc.tile_pool(name="dram", bufs=4, space="DRAM") as dramp,
        ):
            w_t = constp.tile([C, C], bf16, name="w_t")
            nc.sync.dma_start(w_t[:], wmat[:])
            io_t = constp.tile([128, KSEL * W], bf16, name="io_t")
            nc.sync.dma_start(io_t[:], iotaT[:])

            ztbs = []

            def emit_mat(plan, idxT, auxT, dstp, l, srcs, consume_block):
                """Gathers (chunk-interleaved lo/hi, balanced by block
                coverage) + per-block matmul consumption for one (l, mat)."""
                ao0 = int(plan.auxoff[l, 0, 0, 0])
                a_end = (int(plan.auxoff[l, 1, NBPC - 1, 1])
                         + 2 * int(plan.nt[l, 1, NBPC - 1, 1]))
                vt = auxp.tile([128, a_end - ao0], bf16, tag="aux")
                nc.sync.dma_start(vt[:], auxT[:, ao0:a_end])

                QCAP = 4 * CAP
                runs = []
                for h in range(2):
                    s0 = int(plan.segoff[l, h, 0, 0])
                    ln = (int(plan.segoff[l, h, NBPC - 1, 1])
                          + int(plan.slots[l, h, NBPC - 1, 1]) - s0)
                    chunks = []
                    o = 0
                    while o < ln:
                        n = min(CAP, ln - o)
                        chunks.append((s0 + o, n))
                        o += n
                    runs.append(chunks)
                itiles = {}         # (h, qi) -> idx tile

                def idx_tile(h, i, s0, n):
                    # one idx tile covers 4 consecutive chunks (QCAP slots)
                    qi = i // 4
                    if (h, qi) not in itiles:
                        run0 = int(plan.segoff[l, h, 0, 0])
                        qs0 = run0 + qi * QCAP
                        ln = (int(plan.segoff[l, h, NBPC - 1, 1])
                              + int(plan.slots[l, h, NBPC - 1, 1]) - run0)
                        qn = min(QCAP, run0 + ln - qs0)
                        it = idxp.tile([128, QCAP // 16], i16, tag="idx")
                        nc.sync.dma_start(
                            it[:, :qn // 16],
                            idxT[:, qs0 // 16:(qs0 + qn) // 16])
                        itiles[(h, qi)] = (it, qs0)
                    it, qs0 = itiles[(h, qi)]
                    return it[:, (s0 - qs0) // 16:(s0 - qs0 + n) // 16]

                tiles = {}          # (h, i) -> dst tile
                covered = [0, 0]    # absolute slot end gathered per half
                nextb = [0]

                def emit_ready():
                    while nextb[0] < NBPC:
                        b = nextb[0]
                        ends = [int(plan.segoff[l, h, b, 1])
                                + int(plan.slots[l, h, b, 1])
                                for h in range(2)]
                        if covered[0] < ends[0] or covered[1] < ends[1]:
                            return
                        # tiles of this block in (w, h, t) order so each
                        # window forms one contiguous accumulation chain
                        tl = []
                        for w in range(2):
                            for h in range(2):
                                seg0 = int(plan.segoff[l, h, b, w])
                                nt = int(plan.nt[l, h, b, w])
                                run0 = int(plan.segoff[l, h, 0, 0])
                                for t in range(nt):
                                    s = seg0 + t * 128 - run0
                                    ti, loc = s // CAP, (s % CAP) // 128
                                    tl.append((tiles[(h, ti)], loc,
                                               (h, b, w, t)))
                        consume_block(b, tl, vt, ao0)
                        nextb[0] += 1

                def blocks_done(h):
                    bc = 0
                    while bc < NBPC and (int(plan.segoff[l, h, bc, 1])
                                         + int(plan.slots[l, h, bc, 1])
                                         <= covered[h]):
                        bc += 1
                    return bc

                ci = [0, 0]
                while ci[0] < len(runs[0]) or ci[1] < len(runs[1]):
                    if ci[0] >= len(runs[0]):
                        h = 1
                    elif ci[1] >= len(runs[1]):
                        h = 0
                    else:
                        h = 0 if blocks_done(0) <= blocks_done(1) else 1
                    s0, n = runs[h][ci[h]]
                    dst = dstp.tile([128, CAP // 128, C], bf16, tag="dst")
                    iv = idx_tile(h, ci[h], s0, n)
                    if ci[h] % 4 == 2 and ci[h] + 2 < len(runs[h]):
                        ps0, pn = runs[h][ci[h] + 2]
                        idx_tile(h, ci[h] + 2, ps0, pn)
                    nc.gpsimd.dma_gather(
                        dst[:, :n // 128, :], srcs[h],
                        iv, n, n, C,
                        single_packet=False, queue_num=q())
                    tiles[(h, ci[h])] = dst
                    covered[h] = s0 + n
                    ci[h] += 1
                    emit_ready()
                emit_ready()
                assert nextb[0] == NBPC

            def sel_groups(tl, vt, ao0, plan, l):
                """Yield (sel, k, tile, loc, w, first, last): sel columns
                [k*W:(k+1)*W] for consecutive tiles of one sub-segment."""
                nw = [0, 0]
                for _, _, (h, b, w, t) in tl:
                    nw[w] += 1
                done = [0, 0]
                i = 0
                while i < len(tl):
                    h, b, w, t0 = tl[i][2]
                    nt = int(plan.nt[l, h, b, w])
                    K = min(KSEL, nt - t0)
                    ao = int(plan.auxoff[l, h, b, w]) - ao0
                    sel = selp.tile([128, KSEL * W], bf16, tag="sel")
                    s3 = sel[:, :K * W].rearrange("p (k r) -> p k r", k=K)
                    rl_b = vt[:, ao + t0:ao + t0 + K].to_broadcast(
                        [128, K, W])
                    io_v = io_t[:, :K * W].rearrange(
                        "p (k r) -> p k r", k=K)
                    nc.vector.tensor_tensor(out=s3, in0=rl_b, in1=io_v,
                                            op=eq)
                    v_b = vt[:, ao + nt + t0:ao + nt + t0 + K
                             ].to_broadcast([128, K, W])
                    nc.vector.tensor_tensor(out=s3, in0=s3, in1=v_b,
                                            op=mult)
                    for k in range(K):
                        yield (sel, k, tl[i + k][0], tl[i + k][1], w,
                               done[w] == 0, done[w] == nw[w] - 1)
                        done[w] += 1
                    i += K

            def spmm1_scale(l):
                zshw = dramp.tile([128, NBPC * C], bf16, tag="zshw")
                ztbw = dramp.tile([TBL, C], bf16, tag="ztbw",
                                  addr_space="Shared")
                ztbs.append(ztbw)
                zsb = zsbp.tile([128, NBPC * C], bf16, tag="zsb")
                src_lo = featsB[l * N:l * N + HALF, :]
                src_hi = featsB[l * N + HALF:(l + 1) * N, :]

                def consume(b, tl, vt, ao0):
                    psU = psUp.tile([128, 128], f32)
                    for sel, k, dt, loc, w, first, last in sel_groups(
                            tl, vt, ao0, plan1, l):
                        nc.tensor.matmul(
                            psU[:, w * W:(w + 1) * W], lhsT=dt[:, loc, :],
                            rhs=sel[:, k * W:(k + 1) * W],
                            start=first, stop=last)
                    ut = stgp.tile([128, 128], bf16, tag="ut")
                    nc.scalar.activation(ut[:], psU[:], AF.Copy)
                    psZ = psZp.tile([128, 128], f32)
                    nc.tensor.matmul(psZ[:], lhsT=ut[:], rhs=w_t[:],
                                     start=True, stop=True)
                    nc.scalar.activation(zsb[:, b * C:(b + 1) * C], psZ[:],
                                         AF.Copy)

                emit_mat(plan1, idx1, aux1, dst1p, l, (src_lo, src_hi),
                         consume)
                nc.scalar.dma_start(zshw[:], zsb[:])
                nc.gpsimd.collective_compute(
                    "AllGather", mybir.AluOpType.bypass,
                    replica_groups=[list(range(NCORES))],
                    ins=[zshw.opt()], outs=[ztbw.opt()])

            def spmm2_scale(l):
                ztbw = ztbs[l]
                src_lo = ztbw[:HALF, :]
                src_hi = ztbw[HALF:, :]

                def consume(b, tl, vt, ao0):
                    psO = psOp.tile([128, 128], f32)
                    for sel, k, dt, loc, w, first, last in sel_groups(
                            tl, vt, ao0, plan2, l):
                        nc.tensor.matmul(
                            psO[w * W:(w + 1) * W, :],
                            lhsT=sel[:, k * W:(k + 1) * W],
                            rhs=dt[:, loc, :],
                            start=first, stop=last)
                    ot = stgp.tile([128, 128], f32, tag="ot")
                    nc.vector.tensor_copy(ot[:], psO[:])
                    nc.scalar.dma_start(outp[l, b * BLK:(b + 1) * BLK, :],
                                        ot[:])

                emit_mat(plan2, idx2, aux2, dst2p, l, (src_lo, src_hi),
                         consume)

            done1 = done2 = 0
            while done2 < scales:
                if done1 < scales:
                    spmm1_scale(done1)
                    done1 += 1
                if done1 >= min(2, scales) and done2 < done1:
                    spmm2_scale(done2)
                    done2 += 1
    nc.compile()
    return nc


def kernel(**inputs):
    _install_hook_stub()
    from concourse.bass_utils import run_bass_kernel_spmd

    feats = np.asarray(inputs["features"], np.float32)        # [N, L, C]
    featsB = np.ascontiguousarray(
        feats.transpose(1, 0, 2).reshape(L * N, C)).astype(BF16)
    wmat = np.asarray(inputs["weight_matrix"], np.float32).astype(BF16)

    plan1, idx1, aux1, plan2, idx2, aux2 = _preprocess(
        np.asarray(inputs["phi_indices"]), np.asarray(inputs["phi_values"]),
        np.asarray(inputs["phi_inverse_indices"]),
        np.asarray(inputs["phi_inverse_values"]),
        np.asarray(inputs["diagonal_weight_filter"]))

    scales = int(os.environ.get("DGW_SCALES", L))
    nc = _build(plan1, plan2, scales)

    iotaT = np.ascontiguousarray(np.tile(
        np.arange(W, dtype=np.float32)[None, :], (128, KSEL))).astype(BF16)

    in_maps = []
    for c in range(NCORES):
        in_maps.append(dict(
            featsB=featsB, wmat=wmat, iotaT=iotaT,
            idx1=np.ascontiguousarray(idx1[c]),
            idx2=np.ascontiguousarray(idx2[c]),
            aux1=np.ascontiguousarray(aux1[c]),
            aux2=np.ascontiguousarray(aux2[c])))
    res = run_bass_kernel_spmd(nc, in_maps, core_ids=list(range(NCORES)))
    kernel.last_results = res

    shards = np.stack([res.results[c]["outp"] for c in range(NCORES)])
    blocks = shards.reshape(NCORES, L, NBPC, BLK, C).transpose(2, 0, 3, 1, 4)
    out = blocks.reshape(NBPC * NCORES * BLK, L, C)[:N]
    return np.ascontiguousarray(out)
